# revision 50
# baseline (speedup 1.0000x reference)
"""GCN-GRU cell fused Trainium2 kernel (8-core data parallel).

Math (per batch b):
    A = d * (adj+I).T * d,  d = rowsum(adj+I)^-0.5
    conc1 = [input, hidden]                (N, 65)
    sig   = sigmoid(A @ conc1 @ W1 + b1)   (N, 128)  node-major flat
    r, u  = first/second half of flat(sig) -> pseudo-node split
    rh    = r * hidden_flat
    c     = tanh(A @ [input, rh] @ W2 + b2)
    out   = u * hidden_flat + (1-u) * c

Implementation highlights (~2.5x over the bf16 v1 at 275us):
  - Big A@X GEMMs in fp8e4 with perf_mode=DoubleRow: rhs [128, 2kt, 512]
    streams 1024 fp8 columns into a 512-col f32 PSUM accumulator per
    matmul; each lhsT load is reused across 4 chunk accumulators
    (4 reuse-passes per GCN over a shared 4-slot psum ring with 2 slots
    of slack so pass k+1 never waits on pass k's drain copies).
  - Both d factors folded on host: A columns pre-scaled by 32*d[m], X
    rows by 32*d[n] (d ~ 2^-5 so the scaling is ~lossless in fp8); the
    2^10 factor is divided out of the tiny W matrices host-side. The
    PSUM->SBUF drains are then plain dtype casts, split between the DVE
    and scalar engines to balance load.
  - A@input (0.5 GFLOP total) is computed on host and DMA'd into
    dedicated partition rows of the ax work tile, so each W-stage
    output tile is a SINGLE K=128 matmul (feats + axin + bias-ones +
    zero rows) instead of two accumulating matmuls.
  - W-stage psum: one accumulation group per 2KB bank (4 batches/bank,
    start on first, stop on last slice) so sigmoid/tanh run as 24 large
    [128,1024] activations instead of 192 small ones. GCN2 drains into
    a separate ax tile (axw2) so W1's matmul reads never race GCN2's
    copies and the psum ring keeps its 2-slot slack.
  - Gate terms u*h and (1-u) are hoisted into the big-GEMM window as two
    whole-tensor DVE ops; the per-group gate is then 2 bf16 ops.
  - All large inputs AND the output are host-prearranged partition-major
    so every DMA moves multi-KB contiguous runs per partition; A/x1 are
    split first-needed-first across the 3 DMA queues so the PE starts
    after ~0.6MB landed.
"""

import numpy as np
import ml_dtypes
from contextlib import ExitStack

import concourse.bacc as bacc
import concourse.mybir as mybir
import concourse.tile as tile
from concourse.bass import ts, ds
from concourse.bass_utils import run_bass_kernel_spmd

P = 128
N = 2048
B = 64
H = 64
NCORES = 8
BL = B // NCORES          # 8 batches per core
KT = N // P               # 16 contraction tiles
NT = KT // 2              # 8 (pair-tiles / half-node groups)
CH = N // 512             # 4 output chunks of 512
F32 = mybir.dt.float32
BF16 = mybir.dt.bfloat16
FP8 = mybir.dt.float8e4
SIG = mybir.ActivationFunctionType.Sigmoid
TANH = mybir.ActivationFunctionType.Tanh
DR = mybir.MatmulPerfMode.DoubleRow

_CACHE = {}


def _build():
    nc = bacc.Bacc("TRN2", target_bir_lowering=False)

    a_d = nc.dram_tensor("a", [P, KT * N], FP8, kind="ExternalInput")
    x1_d = nc.dram_tensor("x1", [P, KT * BL * H], FP8, kind="ExternalInput")
    hrm_d = nc.dram_tensor("hrm", [P, BL, NT, 2 * H], BF16, kind="ExternalInput")
    axe_d = nc.dram_tensor("axe", [64, BL // 2, N], BF16, kind="ExternalInput")
    axo_d = nc.dram_tensor("axo", [64, BL // 2, N], BF16, kind="ExternalInput")
    w1e_d = nc.dram_tensor("w1e", [P, 2 * H], BF16, kind="ExternalInput")
    w1o_d = nc.dram_tensor("w1o", [P, 2 * H], BF16, kind="ExternalInput")
    w2e_d = nc.dram_tensor("w2e", [P, H], BF16, kind="ExternalInput")
    w2o_d = nc.dram_tensor("w2o", [P, H], BF16, kind="ExternalInput")
    out_d = nc.dram_tensor("out", [P, NT, 2, 4, 2 * H], BF16, kind="ExternalOutput")

    out_ap = out_d.ap()

    with tile.TileContext(nc) as tc, ExitStack() as ctx:
        const = ctx.enter_context(tc.tile_pool(name="const", bufs=1))
        a_sb = const.tile([P, KT, N], FP8)
        x1_sb = const.tile([P, KT, BL * H], FP8)
        x2_sb = const.tile([P, KT, BL * H], FP8)
        axw = const.tile([P, BL, N], BF16)
        axw2 = const.tile([P, BL, N], BF16)
        hrm_sb = const.tile([P, BL, NT, 2 * H], BF16)
        sig_r = const.tile([P, NT * BL, 2 * H], BF16)
        sig_u = const.tile([P, NT * BL, 2 * H], BF16)   # becomes um = 1-u after uh
        uh = const.tile([P, NT, BL, 2 * H], BF16)       # u * h, hoisted gate term
        w1e_sb = const.tile([P, 2 * H], BF16)
        w1o_sb = const.tile([P, 2 * H], BF16)
        w2e_sb = const.tile([P, H], BF16)
        w2o_sb = const.tile([P, H], BF16)

        a_r = a_d.ap().rearrange("p (kt m) -> p kt m", m=N)
        x1_r = x1_d.ap().rearrange("p (kt f) -> p kt f", f=BL * H)
        # first-needed-first, split across 3 queues so the PE starts early
        # and the A supply keeps pace with pass 0's consumption
        nc.sync.dma_start(a_sb[:, 0:2, 0:1024], a_r[:, 0:2, 0:1024])
        nc.scalar.dma_start(x1_sb[:, 0:2, :], x1_r[:, 0:2, :])
        nc.gpsimd.dma_start(a_sb[:, 0:2, 1024:2048], a_r[:, 0:2, 1024:2048])
        nc.scalar.dma_start(x1_sb[:, 2:8, :], x1_r[:, 2:8, :])
        nc.sync.dma_start(a_sb[:, 2:4, :], a_r[:, 2:4, :])
        nc.gpsimd.dma_start(a_sb[:, 4:6, :], a_r[:, 4:6, :])
        nc.scalar.dma_start(x1_sb[:, 8:16, :], x1_r[:, 8:16, :])
        nc.sync.dma_start(a_sb[:, 6:8, :], a_r[:, 6:8, :])
        nc.gpsimd.dma_start(a_sb[:, 8:10, :], a_r[:, 8:10, :])
        nc.sync.dma_start(a_sb[:, 10:12, :], a_r[:, 10:12, :])
        nc.scalar.dma_start(a_sb[:, 12:14, :], a_r[:, 12:14, :])
        nc.sync.dma_start(a_sb[:, 14:16, :], a_r[:, 14:16, :])
        nc.gpsimd.dma_start(w1e_sb[:], w1e_d.ap())
        nc.gpsimd.dma_start(w1o_sb[:], w1o_d.ap())
        nc.gpsimd.dma_start(w2e_sb[:], w2e_d.ap())
        nc.gpsimd.dma_start(w2o_sb[:], w2o_d.ap())
        nc.gpsimd.dma_start(axw[64:128, 0:BL:2, :], axe_d.ap())
        nc.gpsimd.dma_start(axw[0:64, 1:BL:2, :], axo_d.ap())
        nc.gpsimd.dma_start(axw2[64:128, 0:BL:2, :], axe_d.ap())
        nc.gpsimd.dma_start(axw2[0:64, 1:BL:2, :], axo_d.ap())
        nc.gpsimd.dma_start(hrm_sb[:], hrm_d.ap())

        pps = ctx.enter_context(tc.tile_pool(name="ps", bufs=1, space="PSUM"))
        cpool = ctx.enter_context(tc.tile_pool(name="c", bufs=4))
        gpool = ctx.enter_context(tc.tile_pool(name="g", bufs=4))

        def big_pass(xsb, mf, tagpfx, copy_eng, axdst):
            # 2 psum tiles of 2 banks each (chunk pairs); the shared 4-slot
            # ring leaves 2 slots of slack so pass k+1 never waits on pass
            # k's drain copies.
            ps = [
                pps.tile([P, 2, 512], F32, tag="bg", bufs=4, name=f"{tagpfx}{mf}c{cp}")
                for cp in range(2)
            ]
            for tp in range(NT):
                lhsT = xsb[:, 2 * tp : 2 * tp + 2, ts(mf, P)]
                for ch in range(CH):
                    nc.tensor.matmul(
                        ps[ch // 2][:, ch % 2, :],
                        lhsT=lhsT,
                        rhs=a_sb[:, 2 * tp : 2 * tp + 2, ts(ch, 512)],
                        start=(tp == 0), stop=(tp == NT - 1),
                        perf_mode=DR,
                    )
            # psum (= 2^10 * true ax) -> axw feats rows, plain dtype cast.
            # copy engine: "v"/"s"/"b" (split across both for fastest drain)
            for cp in range(2):
                cols = ds(cp * 1024, 1024)
                de = axdst[0:64, 2 * mf, cols].rearrange("p (c m) -> p c m", c=2)
                do = axdst[64:128, 2 * mf + 1, cols].rearrange("p (c m) -> p c m", c=2)
                pse, pso = ps[cp][0:64, :, :], ps[cp][64:128, :, :]
                if copy_eng == "v":
                    nc.vector.tensor_copy(out=de, in_=pse)
                    nc.vector.tensor_copy(out=do, in_=pso)
                elif copy_eng == "s":
                    nc.scalar.copy(de, pse)
                    nc.scalar.copy(do, pso)
                else:
                    nc.scalar.copy(de, pse)
                    nc.vector.tensor_copy(out=do, in_=pso)

        def w2_gate_t(t, split=False):
            # full batch for node group t: 2 psum units, then act / fused
            # gate g = um*c + uh (um, uh hoisted) / store — whole-t when
            # steady-state, half-batch chains for the last unit (shorter
            # critical tail).
            cs = cpool.tile([P, BL, 2 * H], BF16, tag="c")
            ch = t // 2
            pcg = pps.tile([P, 2, 512], F32, tag="bg", bufs=4, name=f"pc{t}")
            for hb in (0, 4):
                pc3 = pcg[:, hb // 4, :].rearrange("p (i f) -> p i f", i=4)
                for i in range(4):
                    b = hb + i
                    rhs = w2e_sb if b % 2 == 0 else w2o_sb
                    for j in (0, 1):
                        lo = 512 * ch + 256 * (t % 2) + j
                        nc.tensor.matmul(
                            pc3[:, i, ds(64 * j, 64)],
                            lhsT=axw2[:, b, lo : lo + 255 : 2],
                            rhs=rhs[:],
                            start=(i == 0 and j == 0), stop=(i == 3 and j == 1),
                        )
                if split:
                    nc.scalar.activation(cs[:, hb : hb + 4, :], pcg[:, hb // 4, :], TANH)
                    um3 = sig_u[:, ds(t * BL + hb, 4), :]
                    uh3 = uh[:, t, hb : hb + 4, :]
                    g = gpool.tile([P, 4, 2 * H], BF16, tag="g2")
                    nc.vector.tensor_mul(g[:], um3, cs[:, hb : hb + 4, :])
                    nc.vector.tensor_add(g[:], g[:], uh3)
                    deng = nc.gpsimd if hb == 0 else nc.sync
                    deng.dma_start(out_ap[:, t, hb // 4, :, :], g[:])
            if not split:
                nc.scalar.activation(cs[:], pcg[:], TANH)
                um3 = sig_u[:, ts(t, BL), :]
                uh3 = uh[:, t, :, :]
                g = gpool.tile([P, BL, 2 * H], BF16, tag="g")
                nc.vector.tensor_mul(g[:], um3, cs[:])
                nc.vector.tensor_add(g[:], g[:], uh3)
                deng = nc.gpsimd if t % 2 == 0 else nc.sync
                deng.dma_start(
                    out_ap[:, t, :, :, :].rearrange("p a b f -> p (a b) f"), g[:]
                )

        # ---- GCN1: 4 reuse-passes, copies on DVE (last split) ----
        for mf in range(4):
            big_pass(x1_sb, mf, "p1m", "v" if mf < 3 else "b", axw)

        def w1_mt(mt):
            # one 128-node group x 8 batches: 8 MMs + one [128,1024] sigmoid
            pm = pps.tile([P, 2, 512], F32, tag="bg", bufs=4, name=f"pm{mt}")
            for h in (0, 1):
                pm3 = pm[:, h, :].rearrange("p (i f) -> p i f", i=4)
                for i in range(4):
                    b = 4 * h + i
                    rhs = w1e_sb if b % 2 == 0 else w1o_sb
                    nc.tensor.matmul(
                        pm3[:, i, :],
                        lhsT=axw[:, b, ts(mt, P)],
                        rhs=rhs[:],
                        start=(i == 0), stop=(i == 3),
                    )
            if mt < NT:
                dst = sig_r[:, ts(mt, BL), :]
            else:
                dst = sig_u[:, ts(mt - NT, BL), :]
            nc.scalar.activation(dst, pm[:], SIG)

        # ---- W1 (all 16 node groups) + sigmoid; x2 assembly (DVE) trails
        # sig_r groups so big2 can start the moment W1's matmuls end ----
        for mt in range(KT):
            w1_mt(mt)
            if mt < NT:
                for kt in (mt, mt + NT):
                    jo = 0 if kt < NT else 64
                    s3 = sig_r[:, ts(mt, BL), jo : jo + 64]
                    x13 = x1_sb[:, kt, :].rearrange("p (b h) -> p b h", h=H)
                    x23 = x2_sb[:, kt, :].rearrange("p (b h) -> p b h", h=H)
                    nc.vector.tensor_mul(x23, s3, x13)

        # hoisted gate terms (per t, run on DVE during GCN2's passes):
        # uh = u*h, then sig_u <- 1-u in place
        for t in range(NT):
            su = sig_u[:, ts(t, BL), :]
            nc.vector.tensor_mul(uh[:, t, :, :], su, hrm_sb[:, :, t, :])
            nc.vector.tensor_scalar(
                out=su, in0=su, scalar1=-1.0, scalar2=1.0,
                op0=mybir.AluOpType.mult, op1=mybir.AluOpType.add,
            )

        # ---- GCN2 passes (drain into axw2; 2-slot ring slack restored) ----
        for mf in range(4):
            big_pass(x2_sb, mf, "p2m", "s" if mf < 3 else "b", axw2)

        for t in range(NT):
            w2_gate_t(t, split=(t == NT - 1))

    nc.finalize()
    return nc


def _prep_inputs(input_tensor, hidden, adj, W1, b1, W2, b2):
    f32 = np.float32
    bf16 = ml_dtypes.bfloat16
    fp8 = ml_dtypes.float8_e4m3
    input_tensor = np.ascontiguousarray(input_tensor, f32)
    hidden = np.ascontiguousarray(hidden, f32)
    adj = np.ascontiguousarray(adj, f32)
    W1 = np.asarray(W1, f32); b1 = np.asarray(b1, f32)
    W2 = np.asarray(W2, f32); b2 = np.asarray(b2, f32)

    pi = np.concatenate([np.arange(0, N, 2), np.arange(1, N, 2)])
    a_hat = adj + np.eye(N, dtype=f32)
    deg = a_hat.sum(axis=1, dtype=np.float64)
    d = (deg ** -0.5).astype(f32)

    # A columns scaled by 32*d[m], rows permuted; partition-major layout
    a_s = (a_hat[pi] * (32.0 * d)[None, :]).astype(fp8)
    a_pre = np.ascontiguousarray(
        a_s.reshape(KT, P, N).transpose(1, 0, 2).reshape(P, KT * N)
    )

    # host A@input: axin[b, m] = 2^10 * d[m] * sum_n a_hat[n,m] d[n] in[b,n]
    din = (d[None, :] * input_tensor).astype(f32)
    axin_s = (1024.0 * (din @ a_hat) * d[None, :]).astype(bf16)     # (B, N)

    w1e = np.zeros((P, 2 * H), bf16)
    w1e[0:64] = (W1[1:] / 1024.0).astype(bf16)
    w1e[64] = (W1[0] / 1024.0).astype(bf16)
    w1e[65] = b1.astype(bf16)
    w1o = np.zeros((P, 2 * H), bf16)
    w1o[62] = (W1[0] / 1024.0).astype(bf16); w1o[63] = b1.astype(bf16)
    w1o[64:128] = (W1[1:] / 1024.0).astype(bf16)
    w2e = np.zeros((P, H), bf16)
    w2e[0:64] = (W2[1:] / 1024.0).astype(bf16)
    w2e[64] = (W2[0] / 1024.0).astype(bf16)
    w2e[65] = b2.astype(bf16)
    w2o = np.zeros((P, H), bf16)
    w2o[62] = (W2[0] / 1024.0).astype(bf16); w2o[63] = b2.astype(bf16)
    w2o[64:128] = (W2[1:] / 1024.0).astype(bf16)

    dh = (32.0 * d[None, :, None] * hidden).astype(f32)             # (B, N, H)

    in_maps = []
    for c in range(NCORES):
        bs = slice(BL * c, BL * c + BL)
        x1n = dh[bs][:, pi, :].transpose(1, 0, 2).reshape(N, BL * H)
        x1 = np.ascontiguousarray(
            x1n.reshape(KT, P, BL * H).transpose(1, 0, 2).reshape(P, KT * BL * H)
        ).astype(fp8)
        hrm = np.ascontiguousarray(
            hidden[bs].reshape(BL, NT, P, 2 * H).transpose(2, 0, 1, 3)
        ).astype(bf16)
        axc = axin_s[bs]                                            # (8, N) bf16
        axe = np.zeros((64, BL // 2, N), bf16)
        axe[0] = axc[0:BL:2]; axe[1] = 1.0
        axo = np.zeros((64, BL // 2, N), bf16)
        axo[62] = axc[1:BL:2]; axo[63] = 1.0
        in_maps.append({
            "a": a_pre, "x1": x1, "hrm": hrm,
            "axe": axe, "axo": axo,
            "w1e": w1e, "w1o": w1o, "w2e": w2e, "w2o": w2o,
        })
    return in_maps


LAST_RESULTS = None


def kernel(input_tensor, hidden, adj, W1, b1, W2, b2):
    global LAST_RESULTS
    if "nc" not in _CACHE:
        _CACHE["nc"] = _build()
    nc = _CACHE["nc"]
    in_maps = _prep_inputs(input_tensor, hidden, adj, W1, b1, W2, b2)
    res = run_bass_kernel_spmd(nc, in_maps, core_ids=list(range(NCORES)))
    LAST_RESULTS = res
    outs = []
    for r in res.results:
        o = np.asarray(r["out"]).astype(np.float32)     # [P, NT, 2, 4, 2H]
        o = o.transpose(2, 3, 1, 0, 4).reshape(BL, NT * P, 2 * H)
        outs.append(o)
    return np.concatenate(outs, axis=0).reshape(B, N, H)


if __name__ == "__main__":
    rng = np.random.default_rng(0)
    inputs = {
        "input_tensor": rng.standard_normal((B, N), dtype=np.float32),
        "hidden": rng.standard_normal((B, N, H), dtype=np.float32),
        "adj": rng.random((N, N), dtype=np.float32),
        "W1": rng.standard_normal((H + 1, 2 * H), dtype=np.float32) * 0.15,
        "b1": np.full((2 * H,), 0.4, np.float32),
        "W2": rng.standard_normal((H + 1, H), dtype=np.float32) * 0.15,
        "b2": np.full((H,), 0.6, np.float32),
    }
    out = kernel(**inputs)
    print(out.shape, out.dtype)


# revision 51
# speedup vs baseline: 1.0080x; 1.0080x over previous
"""GCN-GRU cell fused Trainium2 kernel (8-core data parallel).

Math (per batch b):
    A = d * (adj+I).T * d,  d = rowsum(adj+I)^-0.5
    conc1 = [input, hidden]                (N, 65)
    sig   = sigmoid(A @ conc1 @ W1 + b1)   (N, 128)  node-major flat
    r, u  = first/second half of flat(sig) -> pseudo-node split
    rh    = r * hidden_flat
    c     = tanh(A @ [input, rh] @ W2 + b2)
    out   = u * hidden_flat + (1-u) * c

Implementation highlights (~2.5x over the bf16 v1 at 275us):
  - Big A@X GEMMs in fp8e4 with perf_mode=DoubleRow: rhs [128, 2kt, 512]
    streams 1024 fp8 columns into a 512-col f32 PSUM accumulator per
    matmul; each lhsT load is reused across 4 chunk accumulators
    (4 reuse-passes per GCN over a shared 4-slot psum ring with 2 slots
    of slack so pass k+1 never waits on pass k's drain copies).
  - Both d factors folded on host: A columns pre-scaled by 32*d[m], X
    rows by 32*d[n] (d ~ 2^-5 so the scaling is ~lossless in fp8); the
    2^10 factor is divided out of the tiny W matrices host-side. The
    PSUM->SBUF drains are then plain dtype casts, split between the DVE
    and scalar engines to balance load.
  - A@input (0.5 GFLOP total) is computed on host and DMA'd into
    dedicated partition rows of the ax work tile, so each W-stage
    output tile is a SINGLE K=128 matmul (feats + axin + bias-ones +
    zero rows) instead of two accumulating matmuls.
  - W-stage psum: one accumulation group per 2KB bank (4 batches/bank,
    start on first, stop on last slice) so sigmoid/tanh run as 24 large
    [128,1024] activations instead of 192 small ones. GCN2 drains into
    a separate ax tile (axw2) so W1's matmul reads never race GCN2's
    copies and the psum ring keeps its 2-slot slack.
  - Gate terms u*h and (1-u) are hoisted into the big-GEMM window as two
    whole-tensor DVE ops; the per-group gate is then 2 bf16 ops.
  - All large inputs AND the output are host-prearranged partition-major
    so every DMA moves multi-KB contiguous runs per partition; A/x1 are
    split first-needed-first across the 3 DMA queues so the PE starts
    after ~0.6MB landed.
"""

import numpy as np
import ml_dtypes
from contextlib import ExitStack

import concourse.bacc as bacc
import concourse.mybir as mybir
import concourse.tile as tile
from concourse.bass import ts, ds
from concourse.bass_utils import run_bass_kernel_spmd

P = 128
N = 2048
B = 64
H = 64
NCORES = 8
BL = B // NCORES          # 8 batches per core
KT = N // P               # 16 contraction tiles
NT = KT // 2              # 8 (pair-tiles / half-node groups)
CH = N // 512             # 4 output chunks of 512
F32 = mybir.dt.float32
BF16 = mybir.dt.bfloat16
FP8 = mybir.dt.float8e4
SIG = mybir.ActivationFunctionType.Sigmoid
TANH = mybir.ActivationFunctionType.Tanh
DR = mybir.MatmulPerfMode.DoubleRow

_CACHE = {}


def _build():
    nc = bacc.Bacc("TRN2", target_bir_lowering=False)

    a_d = nc.dram_tensor("a", [P, KT * N], FP8, kind="ExternalInput")
    x1_d = nc.dram_tensor("x1", [P, KT * BL * H], FP8, kind="ExternalInput")
    hrm_d = nc.dram_tensor("hrm", [P, BL, NT, 2 * H], BF16, kind="ExternalInput")
    axe_d = nc.dram_tensor("axe", [64, BL // 2, N], BF16, kind="ExternalInput")
    axo_d = nc.dram_tensor("axo", [64, BL // 2, N], BF16, kind="ExternalInput")
    w1e_d = nc.dram_tensor("w1e", [P, 2 * H], BF16, kind="ExternalInput")
    w1o_d = nc.dram_tensor("w1o", [P, 2 * H], BF16, kind="ExternalInput")
    w2e_d = nc.dram_tensor("w2e", [P, H], BF16, kind="ExternalInput")
    w2o_d = nc.dram_tensor("w2o", [P, H], BF16, kind="ExternalInput")
    out_d = nc.dram_tensor("out", [P, NT, 2, 4, 2 * H], BF16, kind="ExternalOutput")

    out_ap = out_d.ap()

    with tile.TileContext(nc) as tc, ExitStack() as ctx:
        const = ctx.enter_context(tc.tile_pool(name="const", bufs=1))
        a_sb = const.tile([P, KT, N], FP8)
        x1_sb = const.tile([P, KT, BL * H], FP8)
        x2_sb = const.tile([P, KT, BL * H], FP8)
        axw = const.tile([P, BL, N], BF16)
        axw2 = const.tile([P, BL, N], BF16)
        hrm_sb = const.tile([P, BL, NT, 2 * H], BF16)
        sig_r = const.tile([P, NT * BL, 2 * H], BF16)
        sig_u = const.tile([P, NT * BL, 2 * H], BF16)   # becomes um = 1-u after uh
        uh = const.tile([P, NT, BL, 2 * H], BF16)       # u * h, hoisted gate term
        w1e_sb = const.tile([P, 2 * H], BF16)
        w1o_sb = const.tile([P, 2 * H], BF16)
        w2e_sb = const.tile([P, H], BF16)
        w2o_sb = const.tile([P, H], BF16)

        a_r = a_d.ap().rearrange("p (kt m) -> p kt m", m=N)
        x1_r = x1_d.ap().rearrange("p (kt f) -> p kt f", f=BL * H)
        # first-needed-first, split across 3 queues so the PE starts early
        # and the A supply keeps pace with pass 0's consumption
        nc.sync.dma_start(a_sb[:, 0:2, 0:1024], a_r[:, 0:2, 0:1024])
        nc.scalar.dma_start(x1_sb[:, 0:2, :], x1_r[:, 0:2, :])
        nc.gpsimd.dma_start(a_sb[:, 0:2, 1024:2048], a_r[:, 0:2, 1024:2048])
        nc.scalar.dma_start(x1_sb[:, 2:8, :], x1_r[:, 2:8, :])
        nc.sync.dma_start(a_sb[:, 2:4, :], a_r[:, 2:4, :])
        nc.gpsimd.dma_start(a_sb[:, 4:6, :], a_r[:, 4:6, :])
        nc.scalar.dma_start(x1_sb[:, 8:16, :], x1_r[:, 8:16, :])
        nc.sync.dma_start(a_sb[:, 6:9, :], a_r[:, 6:9, :])
        nc.gpsimd.dma_start(a_sb[:, 9:12, :], a_r[:, 9:12, :])
        nc.scalar.dma_start(a_sb[:, 12:14, :], a_r[:, 12:14, :])
        nc.sync.dma_start(a_sb[:, 14:16, :], a_r[:, 14:16, :])
        nc.gpsimd.dma_start(w1e_sb[:], w1e_d.ap())
        nc.gpsimd.dma_start(w1o_sb[:], w1o_d.ap())
        nc.gpsimd.dma_start(w2e_sb[:], w2e_d.ap())
        nc.gpsimd.dma_start(w2o_sb[:], w2o_d.ap())
        nc.gpsimd.dma_start(axw[64:128, 0:BL:2, :], axe_d.ap())
        nc.gpsimd.dma_start(axw[0:64, 1:BL:2, :], axo_d.ap())
        nc.gpsimd.dma_start(axw2[64:128, 0:BL:2, :], axe_d.ap())
        nc.gpsimd.dma_start(axw2[0:64, 1:BL:2, :], axo_d.ap())
        nc.gpsimd.dma_start(hrm_sb[:], hrm_d.ap())

        pps = ctx.enter_context(tc.tile_pool(name="ps", bufs=1, space="PSUM"))
        cpool = ctx.enter_context(tc.tile_pool(name="c", bufs=4))
        gpool = ctx.enter_context(tc.tile_pool(name="g", bufs=4))

        def big_pass(xsb, mf, tagpfx, copy_eng, axdst):
            # 2 psum tiles of 2 banks each (chunk pairs); the shared 4-slot
            # ring leaves 2 slots of slack so pass k+1 never waits on pass
            # k's drain copies.
            ps = [
                pps.tile([P, 2, 512], F32, tag="bg", bufs=4, name=f"{tagpfx}{mf}c{cp}")
                for cp in range(2)
            ]
            for tp in range(NT):
                lhsT = xsb[:, 2 * tp : 2 * tp + 2, ts(mf, P)]
                for ch in range(CH):
                    nc.tensor.matmul(
                        ps[ch // 2][:, ch % 2, :],
                        lhsT=lhsT,
                        rhs=a_sb[:, 2 * tp : 2 * tp + 2, ts(ch, 512)],
                        start=(tp == 0), stop=(tp == NT - 1),
                        perf_mode=DR,
                    )
            # psum (= 2^10 * true ax) -> axw feats rows, plain dtype cast.
            # copy engine: "v"/"s"/"b" (split across both for fastest drain)
            for cp in range(2):
                cols = ds(cp * 1024, 1024)
                de = axdst[0:64, 2 * mf, cols].rearrange("p (c m) -> p c m", c=2)
                do = axdst[64:128, 2 * mf + 1, cols].rearrange("p (c m) -> p c m", c=2)
                pse, pso = ps[cp][0:64, :, :], ps[cp][64:128, :, :]
                if copy_eng == "v":
                    nc.vector.tensor_copy(out=de, in_=pse)
                    nc.vector.tensor_copy(out=do, in_=pso)
                elif copy_eng == "s":
                    nc.scalar.copy(de, pse)
                    nc.scalar.copy(do, pso)
                else:
                    nc.scalar.copy(de, pse)
                    nc.vector.tensor_copy(out=do, in_=pso)

        def w2_gate_t(t, split=False):
            # full batch for node group t: 2 psum units, then act / fused
            # gate g = um*c + uh (um, uh hoisted) / store — whole-t when
            # steady-state, half-batch chains for the last unit (shorter
            # critical tail).
            cs = cpool.tile([P, BL, 2 * H], BF16, tag="c")
            ch = t // 2
            pcg = pps.tile([P, 2, 512], F32, tag="bg", bufs=4, name=f"pc{t}")
            for hb in (0, 4):
                pc3 = pcg[:, hb // 4, :].rearrange("p (i f) -> p i f", i=4)
                for i in range(4):
                    b = hb + i
                    rhs = w2e_sb if b % 2 == 0 else w2o_sb
                    for j in (0, 1):
                        lo = 512 * ch + 256 * (t % 2) + j
                        nc.tensor.matmul(
                            pc3[:, i, ds(64 * j, 64)],
                            lhsT=axw2[:, b, lo : lo + 255 : 2],
                            rhs=rhs[:],
                            start=(i == 0 and j == 0), stop=(i == 3 and j == 1),
                        )
                if split:
                    nc.scalar.activation(cs[:, hb : hb + 4, :], pcg[:, hb // 4, :], TANH)
                    um3 = sig_u[:, ds(t * BL + hb, 4), :]
                    uh3 = uh[:, t, hb : hb + 4, :]
                    g = gpool.tile([P, 4, 2 * H], BF16, tag="g2")
                    nc.vector.tensor_mul(g[:], um3, cs[:, hb : hb + 4, :])
                    nc.vector.tensor_add(g[:], g[:], uh3)
                    deng = nc.gpsimd if hb == 0 else nc.sync
                    deng.dma_start(out_ap[:, t, hb // 4, :, :], g[:])
            if not split:
                nc.scalar.activation(cs[:], pcg[:], TANH)
                um3 = sig_u[:, ts(t, BL), :]
                uh3 = uh[:, t, :, :]
                g = gpool.tile([P, BL, 2 * H], BF16, tag="g")
                nc.vector.tensor_mul(g[:], um3, cs[:])
                nc.vector.tensor_add(g[:], g[:], uh3)
                deng = nc.gpsimd if t % 2 == 0 else nc.sync
                deng.dma_start(
                    out_ap[:, t, :, :, :].rearrange("p a b f -> p (a b) f"), g[:]
                )

        # ---- GCN1: 4 reuse-passes, copies on DVE (last split) ----
        for mf in range(4):
            big_pass(x1_sb, mf, "p1m", "v" if mf < 3 else "b", axw)

        def w1_mt(mt):
            # one 128-node group x 8 batches: 8 MMs + one [128,1024] sigmoid
            pm = pps.tile([P, 2, 512], F32, tag="bg", bufs=4, name=f"pm{mt}")
            for h in (0, 1):
                pm3 = pm[:, h, :].rearrange("p (i f) -> p i f", i=4)
                for i in range(4):
                    b = 4 * h + i
                    rhs = w1e_sb if b % 2 == 0 else w1o_sb
                    nc.tensor.matmul(
                        pm3[:, i, :],
                        lhsT=axw[:, b, ts(mt, P)],
                        rhs=rhs[:],
                        start=(i == 0), stop=(i == 3),
                    )
            if mt < NT:
                dst = sig_r[:, ts(mt, BL), :]
            else:
                dst = sig_u[:, ts(mt - NT, BL), :]
            nc.scalar.activation(dst, pm[:], SIG)

        # ---- W1 (all 16 node groups) + sigmoid; x2 assembly (DVE) trails
        # sig_r groups so big2 can start the moment W1's matmuls end ----
        for mt in range(KT):
            w1_mt(mt)
            if mt < NT:
                for kt in (mt, mt + NT):
                    jo = 0 if kt < NT else 64
                    s3 = sig_r[:, ts(mt, BL), jo : jo + 64]
                    x13 = x1_sb[:, kt, :].rearrange("p (b h) -> p b h", h=H)
                    x23 = x2_sb[:, kt, :].rearrange("p (b h) -> p b h", h=H)
                    nc.vector.tensor_mul(x23, s3, x13)

        # hoisted gate terms (per t, run on DVE during GCN2's passes):
        # uh = u*h, then sig_u <- 1-u in place
        for t in range(NT):
            su = sig_u[:, ts(t, BL), :]
            nc.vector.tensor_mul(uh[:, t, :, :], su, hrm_sb[:, :, t, :])
            nc.vector.tensor_scalar(
                out=su, in0=su, scalar1=-1.0, scalar2=1.0,
                op0=mybir.AluOpType.mult, op1=mybir.AluOpType.add,
            )

        # ---- GCN2 passes (drain into axw2; 2-slot ring slack restored) ----
        for mf in range(4):
            big_pass(x2_sb, mf, "p2m", "s" if mf < 3 else "b", axw2)

        for t in range(NT):
            w2_gate_t(t, split=(t == NT - 1))

    nc.finalize()
    return nc


def _prep_inputs(input_tensor, hidden, adj, W1, b1, W2, b2):
    f32 = np.float32
    bf16 = ml_dtypes.bfloat16
    fp8 = ml_dtypes.float8_e4m3
    input_tensor = np.ascontiguousarray(input_tensor, f32)
    hidden = np.ascontiguousarray(hidden, f32)
    adj = np.ascontiguousarray(adj, f32)
    W1 = np.asarray(W1, f32); b1 = np.asarray(b1, f32)
    W2 = np.asarray(W2, f32); b2 = np.asarray(b2, f32)

    pi = np.concatenate([np.arange(0, N, 2), np.arange(1, N, 2)])
    a_hat = adj + np.eye(N, dtype=f32)
    deg = a_hat.sum(axis=1, dtype=np.float64)
    d = (deg ** -0.5).astype(f32)

    # A columns scaled by 32*d[m], rows permuted; partition-major layout
    a_s = (a_hat[pi] * (32.0 * d)[None, :]).astype(fp8)
    a_pre = np.ascontiguousarray(
        a_s.reshape(KT, P, N).transpose(1, 0, 2).reshape(P, KT * N)
    )

    # host A@input: axin[b, m] = 2^10 * d[m] * sum_n a_hat[n,m] d[n] in[b,n]
    din = (d[None, :] * input_tensor).astype(f32)
    axin_s = (1024.0 * (din @ a_hat) * d[None, :]).astype(bf16)     # (B, N)

    w1e = np.zeros((P, 2 * H), bf16)
    w1e[0:64] = (W1[1:] / 1024.0).astype(bf16)
    w1e[64] = (W1[0] / 1024.0).astype(bf16)
    w1e[65] = b1.astype(bf16)
    w1o = np.zeros((P, 2 * H), bf16)
    w1o[62] = (W1[0] / 1024.0).astype(bf16); w1o[63] = b1.astype(bf16)
    w1o[64:128] = (W1[1:] / 1024.0).astype(bf16)
    w2e = np.zeros((P, H), bf16)
    w2e[0:64] = (W2[1:] / 1024.0).astype(bf16)
    w2e[64] = (W2[0] / 1024.0).astype(bf16)
    w2e[65] = b2.astype(bf16)
    w2o = np.zeros((P, H), bf16)
    w2o[62] = (W2[0] / 1024.0).astype(bf16); w2o[63] = b2.astype(bf16)
    w2o[64:128] = (W2[1:] / 1024.0).astype(bf16)

    dh = (32.0 * d[None, :, None] * hidden).astype(f32)             # (B, N, H)

    in_maps = []
    for c in range(NCORES):
        bs = slice(BL * c, BL * c + BL)
        x1n = dh[bs][:, pi, :].transpose(1, 0, 2).reshape(N, BL * H)
        x1 = np.ascontiguousarray(
            x1n.reshape(KT, P, BL * H).transpose(1, 0, 2).reshape(P, KT * BL * H)
        ).astype(fp8)
        hrm = np.ascontiguousarray(
            hidden[bs].reshape(BL, NT, P, 2 * H).transpose(2, 0, 1, 3)
        ).astype(bf16)
        axc = axin_s[bs]                                            # (8, N) bf16
        axe = np.zeros((64, BL // 2, N), bf16)
        axe[0] = axc[0:BL:2]; axe[1] = 1.0
        axo = np.zeros((64, BL // 2, N), bf16)
        axo[62] = axc[1:BL:2]; axo[63] = 1.0
        in_maps.append({
            "a": a_pre, "x1": x1, "hrm": hrm,
            "axe": axe, "axo": axo,
            "w1e": w1e, "w1o": w1o, "w2e": w2e, "w2o": w2o,
        })
    return in_maps


LAST_RESULTS = None


def kernel(input_tensor, hidden, adj, W1, b1, W2, b2):
    global LAST_RESULTS
    if "nc" not in _CACHE:
        _CACHE["nc"] = _build()
    nc = _CACHE["nc"]
    in_maps = _prep_inputs(input_tensor, hidden, adj, W1, b1, W2, b2)
    res = run_bass_kernel_spmd(nc, in_maps, core_ids=list(range(NCORES)))
    LAST_RESULTS = res
    outs = []
    for r in res.results:
        o = np.asarray(r["out"]).astype(np.float32)     # [P, NT, 2, 4, 2H]
        o = o.transpose(2, 3, 1, 0, 4).reshape(BL, NT * P, 2 * H)
        outs.append(o)
    return np.concatenate(outs, axis=0).reshape(B, N, H)


if __name__ == "__main__":
    rng = np.random.default_rng(0)
    inputs = {
        "input_tensor": rng.standard_normal((B, N), dtype=np.float32),
        "hidden": rng.standard_normal((B, N, H), dtype=np.float32),
        "adj": rng.random((N, N), dtype=np.float32),
        "W1": rng.standard_normal((H + 1, 2 * H), dtype=np.float32) * 0.15,
        "b1": np.full((2 * H,), 0.4, np.float32),
        "W2": rng.standard_normal((H + 1, H), dtype=np.float32) * 0.15,
        "b2": np.full((H,), 0.6, np.float32),
    }
    out = kernel(**inputs)
    print(out.shape, out.dtype)
